# revision 12
# baseline (speedup 1.0000x reference)
"""Trainium2 Bass kernel for nn_Encoder (MPNN: NNConv + GRU + Set2Set).

Self-contained: host-side integer preprocessing (sharding, canonical
orders, slot maps, exchange schedule) + an 8-core SPMD Bass/Tile kernel.

Key structure exploited: edge_attr is all-ones, so the edge-conditioned
weight matrix is identical for every edge and message passing collapses to
   agg = (segment_sum(out[src], dst) * inv_deg) @ W_e.
The segment_sum is computed with on-chip (GPSIMD) gathers over a
degree-bucketed canonical node order; the cross-core rows are exchanged
with one AllToAll per iteration.

Sharding: graphs (batch segments) are split 32-per-core; each core owns its
graphs' nodes. dim-64 parameters are replicated. Segment softmax (Set2Set)
is fully core-local via one-hot matmuls.
"""
import sys
import types
import numpy as np

# ---------------------------------------------------------------------------
# axon NTFF profile hook shim (lets trace=True work when caller requests it)
try:
    import antenv  # noqa: F401
    if 'antenv.axon_hooks' not in sys.modules:
        try:
            from trn_agent_boot.trn_boot import _ntff_profile_via_ctypes
            _hook = _ntff_profile_via_ctypes('/opt/axon/libaxon_pjrt.so')
        except Exception:
            _hook = None
        _m = types.ModuleType('antenv.axon_hooks')
        _m.get_axon_ntff_profile_hook = lambda: _hook
        _m.set_axon_ntff_profile_hook = lambda h: None
        sys.modules['antenv.axon_hooks'] = _m
except ImportError:
    pass

import ml_dtypes
import concourse.bass as bass
import concourse.mybir as mybir
from concourse import bacc
from concourse.bass import get_trn_type
from concourse.tile import TileContext
from concourse import bass_utils
from concourse.masks import make_identity

C = 8
B = 256
DIM = 64
F_IN = 32
GPC = B // C
P = 128
f32 = np.float32
bf16 = ml_dtypes.bfloat16


# ---------------------------------------------------------------------------
# host preprocessing (pure integer / layout work)

def _preprocess(edge_index, batch, n_nodes):
    src_g = np.asarray(edge_index[0], np.int64)
    dst_g = np.asarray(edge_index[1], np.int64)
    batch = np.asarray(batch, np.int64)
    N = n_nodes

    core_starts = np.searchsorted(batch, np.arange(0, B + 1, GPC))
    node_core = batch // GPC
    n_c = np.diff(core_starts)

    deg = np.zeros(N, np.int64)
    np.add.at(deg, dst_g, 1)

    T = int(np.ceil(n_c.max() / 128))
    NPAD = T * 128
    canon = -np.ones((C, NPAD), np.int64)
    pos_of = -np.ones(N, np.int64)
    for c in range(C):
        ids = np.arange(core_starts[c], core_starts[c + 1])
        ids = ids[np.argsort(deg[ids], kind="stable")]
        canon[c, NPAD - len(ids):] = ids
        pos_of[ids] = np.arange(NPAD - len(ids), NPAD)

    deg_canon = np.zeros((C, NPAD), np.int64)
    for c in range(C):
        m = canon[c] >= 0
        deg_canon[c, m] = deg[canon[c, m]]

    D_t = deg_canon.reshape(C, T, 128).max(axis=(0, 2))
    n_slots = int((128 * D_t).sum())
    slot_off = np.concatenate([[0], np.cumsum(128 * D_t)]).astype(np.int64)

    slot_src = -np.ones((C, n_slots), np.int64)
    for c in range(C):
        e_mask = node_core[dst_g] == c
        e_dst_col = pos_of[dst_g[e_mask]]
        e_src = src_g[e_mask]
        order = np.argsort(e_dst_col, kind="stable")
        e_dst_col = e_dst_col[order]
        e_src = e_src[order]
        jj = np.arange(len(e_dst_col)) - np.searchsorted(e_dst_col, e_dst_col)
        t = e_dst_col // 128
        p = e_dst_col % 128
        slot_src[c, slot_off[t] + p * D_t[t] + jj] = e_src

    blocks = [[None] * C for _ in range(C)]
    for c in range(C):
        srcs = slot_src[c]
        srcs = srcs[srcs >= 0]
        for o in range(C):
            blocks[o][c] = np.unique(srcs[node_core[srcs] == o])
    BLK = int(max(len(blocks[o][c]) for o in range(C) for c in range(C)))
    BLK = ((BLK + 63) // 64) * 64

    og_idx = np.zeros((C, C * BLK), np.int64)
    for o in range(C):
        for c in range(C):
            cols = pos_of[blocks[o][c]]
            og_idx[o, c * BLK:c * BLK + len(cols)] = cols

    ZERO_COL = C * BLK
    slotmap = np.full((C, n_slots), ZERO_COL, np.int64)
    for c in range(C):
        for o in range(C):
            blk = blocks[o][c]
            if len(blk) == 0:
                continue
            mask = slot_src[c] >= 0
            s = slot_src[c][mask]
            sel = node_core[s] == o
            ranks = np.searchsorted(blk, s[sel])
            slotmap[c, np.nonzero(mask)[0][sel]] = o * BLK + ranks

    # degree runs: list of (deg, t0, t1) covering tiles with equal D_t
    runs = []
    t0 = 0
    for t in range(1, T + 1):
        if t == T or D_t[t] != D_t[t0]:
            runs.append((int(D_t[t0]), t0, t))
            t0 = t

    return dict(
        T=T, NPAD=NPAD, BLK=BLK, n_slots=n_slots, D_t=D_t, slot_off=slot_off,
        canon=canon, pos_of=pos_of, deg_canon=deg_canon, og_idx=og_idx,
        slotmap=slotmap, ZERO_COL=ZERO_COL, runs=runs, core_starts=core_starts,
    )


def _wrap_idx16(idx, channels=64):
    """ap_gather index layout: element i -> [16*g + i%16, i//16], replicated
    across every 16-partition group."""
    n = len(idx)
    cols = (n + 15) // 16
    flat = np.zeros(cols * 16, np.int64)
    flat[:n] = idx
    base = flat.reshape(cols, 16).T.astype(np.int16)
    out = np.zeros((channels, cols), np.int16)
    for g in range(channels // 16):
        out[16 * g:16 * g + 16] = base
    return out


# ---------------------------------------------------------------------------
# kernel builder

DEBUG_TAPS = False
TRACE = False


def _build(meta):
    T, NPAD, BLK = meta['T'], meta['NPAD'], meta['BLK']
    runs, slot_off = meta['runs'], meta['slot_off']
    OGN = C * BLK
    RCVW = C * BLK + 4
    NCHUNK = (NPAD + 511) // 512
    dt = mybir.dt

    nc = bacc.Bacc(get_trn_type(), target_bir_lowering=False, debug=False,
                   num_devices=C)

    # ---- DRAM I/O ----
    di = lambda name, shape, d=dt.float32: nc.dram_tensor(name, shape, d, kind="ExternalInput")
    x_fm_d = di("x_fm", [F_IN, NPAD])
    inv_d = di("inv_fm", [1, NPAD])
    onehotT_d = di("onehotT", [GPC, NPAD], dt.bfloat16)
    ohnm_d = di("ohnm", [P, T * GPC], dt.bfloat16)
    lin0w_d = di("lin0w", [F_IN, DIM])
    lin0b_d = di("lin0b", [DIM, 1])
    nn1w_d = di("nn1w", [DIM, 1])
    nn1b_d = di("nn1b", [DIM, 1])
    nn2wT_d = di("nn2wT", [DIM, DIM * DIM])
    nn2b_d = di("nn2b", [1, DIM * DIM])
    convb_d = di("convb", [DIM, 1])
    rzlhsT_d = di("rzlhsT", [P, P])
    nlhsT_d = di("nlhsT", [P, P])
    bih3_d = di("bih3", [DIM, 3])
    bhh3_d = di("bhh3", [DIM, 3])
    lstmih_d = di("lstmih", [P, 4 * DIM])
    lstmhh_d = di("lstmhh", [DIM, 4 * DIM])
    lbi4_d = di("lbi4", [DIM, 4])
    lbh4_d = di("lbh4", [DIM, 4])
    og16_d = di("og16", [DIM, OGN // 16], dt.int16)
    sm16_d = di("sm16", [DIM, meta['n_slots'] // 16], dt.int16)

    qs_out_d = nc.dram_tensor("qs_out", [GPC, P], dt.float32, kind="ExternalOutput")
    fm_out_d = nc.dram_tensor("fm_out", [NPAD, DIM], dt.float32, kind="ExternalOutput")
    if DEBUG_TAPS:
        dbg_W_d = nc.dram_tensor("dbg_W", [DIM, DIM], dt.float32, kind="ExternalOutput")
        dbg_h0_d = nc.dram_tensor("dbg_h0", [DIM, NPAD], dt.float32, kind="ExternalOutput")
        dbg_S0_d = nc.dram_tensor("dbg_S0", [DIM, NPAD], dt.float32, kind="ExternalOutput")
        dbg_h1_d = nc.dram_tensor("dbg_h1", [DIM, NPAD], dt.float32, kind="ExternalOutput")
        dbg_rcv0_d = nc.dram_tensor("dbg_rcv0", [DIM, C * BLK + 4], dt.float32, kind="ExternalOutput")

    rg = [list(range(C))]
    AF = mybir.ActivationFunctionType
    AX = mybir.AxisListType
    AL = mybir.AluOpType

    with TileContext(nc) as tc:
        with tc.tile_pool(name="persist", bufs=1) as pp, \
             tc.tile_pool(name="chunk", bufs=2) as cp, \
             tc.tile_pool(name="dram", bufs=1, space="DRAM") as dp:

            # ---------------- static loads ----------------
            def load(pool, d_ap, shape, dtype=dt.float32, tag=None):
                t = pool.tile(shape, dtype, tag=tag or d_ap.name)
                nc.sync.dma_start(out=t[:], in_=d_ap[:])
                return t

            # mh: [0:64] = h state (gather source), [64:128] = m scratch
            mh = pp.tile([P, NPAD], dt.float32, tag="mh")
            onehotT_sb = load(pp, onehotT_d, [GPC, NPAD], dt.bfloat16)
            ohnm_sb = load(pp, ohnm_d, [P, T * GPC], dt.bfloat16)
            lin0w_sb = load(pp, lin0w_d, [F_IN, DIM])
            lin0b_sb = load(pp, lin0b_d, [DIM, 1])
            nn1w_sb = load(pp, nn1w_d, [DIM, 1])
            nn1b_sb = load(pp, nn1b_d, [DIM, 1])
            rzlhsT_sb = load(pp, rzlhsT_d, [P, P])
            nlhsT_sb = load(pp, nlhsT_d, [P, P])
            bih3_sb = load(pp, bih3_d, [DIM, 3])
            bhh3_sb = load(pp, bhh3_d, [DIM, 3])
            lstmih_sb = load(pp, lstmih_d, [P, 4 * DIM])
            lstmhh_sb = load(pp, lstmhh_d, [DIM, 4 * DIM])
            lbi4_sb = load(pp, lbi4_d, [DIM, 4])
            lbh4_sb = load(pp, lbh4_d, [DIM, 4])
            og_sb = load(pp, og16_d, [DIM, OGN // 16], dt.int16)
            sm_sb = load(pp, sm16_d, [DIM, meta['n_slots'] // 16], dt.int16)

            ident = pp.tile([P, P], dt.float32, tag="ident")
            make_identity(nc, ident[:])

            # convb at base partition 64 (conv relu writes mh[64:128])
            convb128 = pp.tile([P, 1], dt.float32, tag="convb128")
            nc.sync.dma_start(out=convb128[0:64, :], in_=convb_d[:])
            nc.sync.dma_start(out=convb128[64:128, :], in_=convb_d[:])

            # combined GRU biases (all base-0 [64, k])
            brz2 = pp.tile([DIM, 2], dt.float32, tag="brz2")
            nc.vector.tensor_add(brz2[:], bih3_sb[:, 0:2], bhh3_sb[:, 0:2])
            lb4 = pp.tile([DIM, 4], dt.float32, tag="lb4")
            nc.vector.tensor_add(lb4[:], lbi4_sb[:], lbh4_sb[:])

            W_sb = pp.tile([DIM, DIM], dt.float32, tag="W_sb")
            inv_bc = pp.tile([DIM, NPAD], dt.float32, tag="inv_bc")
            nc.sync.dma_start(out=inv_bc[0:1, :], in_=inv_d[:])
            nc.gpsimd.partition_broadcast(inv_bc[:], inv_bc[0:1, :], channels=DIM)

            # ---------------- init phase: W_e + lin0 ----------------
            with tc.tile_pool(name="init", bufs=1) as ip, \
                 tc.tile_pool(name="pinit", bufs=2, space="PSUM") as pip:
                x_sb = load(ip, x_fm_d, [F_IN, NPAD])
                nn2wT_sb = load(ip, nn2wT_d, [DIM, DIM * DIM])
                nn2b_sb = load(ip, nn2b_d, [1, DIM * DIM])
                hid = ip.tile([DIM, 1], dt.float32, tag="hid")
                nc.scalar.activation(hid[:], nn1w_sb[:], AF.Relu, bias=nn1b_sb[:, 0:1])
                wvec = ip.tile([1, DIM * DIM], dt.float32, tag="wvec")
                for k in range(8):
                    pw = pip.tile([1, 512], dt.float32, tag="pw")
                    nc.tensor.matmul(pw[:], hid[:],
                                     nn2wT_sb[:, 512 * k:512 * (k + 1)],
                                     start=True, stop=True)
                    nc.vector.tensor_add(wvec[:, 512 * k:512 * (k + 1)], pw[:],
                                         nn2b_sb[:, 512 * k:512 * (k + 1)])
                wbuf = dp.tile([DIM, DIM], dt.float32, tag="wbuf")
                nc.sync.dma_start(out=wbuf[:].rearrange("a b -> (a b)").unsqueeze(0), in_=wvec[:])
                nc.sync.dma_start(out=W_sb[:], in_=wbuf[:])
                if DEBUG_TAPS:
                    nc.sync.dma_start(out=dbg_W_d[:], in_=W_sb[:])

                for ck in range(NCHUNK):
                    sl = slice(512 * ck, min(512 * (ck + 1), NPAD))
                    pl = pip.tile([DIM, 512], dt.float32, tag="pw")
                    w = sl.stop - sl.start
                    nc.tensor.matmul(pl[:, :w], lin0w_sb[:], x_sb[:, sl],
                                     start=True, stop=True)
                    nc.scalar.activation(mh[0:64, sl], pl[:, :w], AF.Relu,
                                         bias=lin0b_sb[:, 0:1])

            if DEBUG_TAPS:
                nc.sync.dma_start(out=dbg_h0_d[:], in_=mh[0:64, :])
            # ---------------- 3 message-passing + GRU iterations ----------------
            with tc.tile_pool(name="work", bufs=1) as wp, \
                 tc.tile_pool(name="ploop", bufs=2, space="PSUM") as psp:
                for it in range(3):
                    staged = wp.tile([DIM, RCVW], dt.float32, tag="xchg")
                    nc.gpsimd.ap_gather(
                        out_ap=staged[:, :OGN].rearrange("f (n o) -> f n o", o=1),
                        in_ap=mh[0:64, :].rearrange("f (n o) -> f n o", o=1),
                        idxs_ap=og_sb[:],
                        channels=DIM, num_elems=NPAD, d=1, num_idxs=OGN,
                    )
                    a2a_in = dp.tile([C, DIM, BLK], dt.float32, tag="a2a_in")
                    nc.sync.dma_start(out=a2a_in[:].rearrange("c f b -> f c b"),
                                      in_=staged[:, :OGN].rearrange("f (c b) -> f c b", b=BLK))
                    a2a_out = dp.tile([C, DIM, BLK], dt.float32, tag=f"a2a_out{it}")
                    nc.gpsimd.collective_compute(
                        "AllToAll", AL.bypass, replica_groups=rg,
                        ins=[a2a_in[:]], outs=[a2a_out[:]],
                    )
                    rcv = wp.tile([DIM, RCVW], dt.float32, tag="xchg")
                    nc.vector.memset(rcv[:, C * BLK:], 0.0)
                    nc.sync.dma_start(out=rcv[:, :C * BLK].rearrange("f (c b) -> f c b", b=BLK),
                                      in_=a2a_out[:].rearrange("c f b -> f c b"))

                    # slot-gather + per-degree-run reduce -> S (feature-major)
                    S = wp.tile([DIM, NPAD], dt.float32, tag="S")
                    for (dg, t0, t1) in runs:
                        ssl = slice(128 * t0, 128 * t1)
                        if dg == 0:
                            nc.vector.memset(S[:, ssl], 0.0)
                            continue
                        nsl = 128 * dg * (t1 - t0)
                        off = int(slot_off[t0])
                        mr = wp.tile([DIM, nsl], dt.float32, tag="mrun")
                        nc.gpsimd.ap_gather(
                            out_ap=mr[:].rearrange("f (n o) -> f n o", o=1),
                            in_ap=rcv[:].rearrange("f (n o) -> f n o", o=1),
                            idxs_ap=sm_sb[:, off // 16:(off + nsl) // 16],
                            channels=DIM, num_elems=RCVW, d=1, num_idxs=nsl,
                        )
                        if dg == 1:
                            nc.vector.tensor_copy(S[:, ssl], mr[:])
                        else:
                            nc.vector.tensor_reduce(
                                S[:, ssl],
                                mr[:].rearrange("f (n j) -> f n j", j=dg),
                                axis=AX.X, op=AL.add)

                    if DEBUG_TAPS and it == 0:
                        nc.sync.dma_start(out=dbg_S0_d[:], in_=S[:])
                        nc.sync.dma_start(out=dbg_rcv0_d[:], in_=rcv[:])
                    # conv + GRU per 512-chunk
                    for ck in range(NCHUNK):
                        sl = slice(512 * ck, min(512 * (ck + 1), NPAD))
                        w = sl.stop - sl.start
                        pc = psp.tile([DIM, 512], dt.float32, tag="pconv")
                        nc.tensor.matmul(pc[:, :w], W_sb[:], S[:, sl],
                                         start=True, stop=True)
                        pc2 = psp.tile([DIM, 512], dt.float32, tag="pconv2")
                        nc.vector.tensor_mul(pc2[:, :w], pc[:, :w], inv_bc[:, sl])
                        nc.scalar.activation(mh[64:128, sl], pc2[:, :w], AF.Relu,
                                             bias=convb128[64:128, :])
                        prz = psp.tile([P, 512], dt.float32, tag="prz")
                        nc.tensor.matmul(prz[:, :w], rzlhsT_sb[:], mh[:, sl],
                                         start=True, stop=True)
                        pn = psp.tile([P, 512], dt.float32, tag="pn")
                        nc.tensor.matmul(pn[:, :w], nlhsT_sb[:], mh[:, sl],
                                         start=True, stop=True)
                        r_t = cp.tile([DIM, 512], dt.float32, tag="r_t")
                        nc.scalar.activation(r_t[:, :w], prz[0:64, :w], AF.Sigmoid,
                                             bias=brz2[:, 0:1])
                        z_t = cp.tile([DIM, 512], dt.float32, tag="z_t")
                        nc.scalar.activation(z_t[:, :w], prz[64:128, :w], AF.Sigmoid,
                                             bias=brz2[:, 1:2])
                        hn_t = cp.tile([DIM, 512], dt.float32, tag="hn_t")
                        nc.scalar.activation(hn_t[:, :w], pn[0:64, :w], AF.Identity,
                                             bias=bhh3_sb[:, 2:3])
                        in_t = cp.tile([DIM, 512], dt.float32, tag="in_t")
                        nc.scalar.activation(in_t[:, :w], pn[64:128, :w], AF.Identity,
                                             bias=bih3_sb[:, 2:3])
                        rn = cp.tile([DIM, 512], dt.float32, tag="rn")
                        nc.vector.tensor_mul(rn[:, :w], r_t[:, :w], hn_t[:, :w])
                        nc.vector.tensor_add(rn[:, :w], rn[:, :w], in_t[:, :w])
                        ng = cp.tile([DIM, 512], dt.float32, tag="ng")
                        nc.scalar.activation(ng[:, :w], rn[:, :w], AF.Tanh)
                        dd = cp.tile([DIM, 512], dt.float32, tag="dd")
                        nc.vector.tensor_sub(dd[:, :w], mh[0:64, sl], ng[:, :w])
                        nc.vector.tensor_mul(dd[:, :w], z_t[:, :w], dd[:, :w])
                        nc.vector.tensor_add(mh[0:64, sl], ng[:, :w], dd[:, :w])
                    if DEBUG_TAPS and it == 0:
                        nc.sync.dma_start(out=dbg_h1_d[:], in_=mh[0:64, :])

            # ---------------- Set2Set ----------------
            with tc.tile_pool(name="s2s", bufs=1) as sp, \
                 tc.tile_pool(name="ps2sA", bufs=2, space="PSUM") as psA, \
                 tc.tile_pool(name="ps2sB", bufs=1, space="PSUM") as psB:

                out_nm = sp.tile([P, T * 65], dt.float32, tag="out_nm")
                nc.vector.memset(
                    out_nm[:].rearrange("p (t w) -> p t w", w=65)[:, :, 64:65], 1.0)
                for t in range(T):
                    pt = psA.tile([P, DIM], dt.float32, tag="psmall")
                    nc.tensor.transpose(pt[:], mh[0:64, 128 * t:128 * (t + 1)],
                                        ident[0:64, 0:64])
                    nc.vector.tensor_copy(out_nm[:, 65 * t:65 * t + 64], pt[:])

                nc.sync.dma_start(
                    out=fm_out_d[:].rearrange("(t p) w -> p t w", p=128),
                    in_=out_nm[:].rearrange("p (t w) -> p t w", w=65)[:, :, 0:64])

                hl = sp.tile([DIM, GPC], dt.float32, tag="hl")
                cl = sp.tile([DIM, GPC], dt.float32, tag="cl")
                qs = sp.tile([P, GPC], dt.float32, tag="qs")
                nc.vector.memset(hl[:], 0.0)
                nc.vector.memset(cl[:], 0.0)
                nc.vector.memset(qs[:], 0.0)
                T4 = ((T + 3) // 4) * 4
                e_nm = sp.tile([P, T4], dt.float32, tag="e_nm")
                nc.vector.memset(e_nm[:], 0.0)
                aout = sp.tile([P, T * 65], dt.bfloat16, tag="aout")

                for step in range(3):
                    # LSTM cell
                    pg1 = psA.tile([P, GPC], dt.float32, tag="psmall")
                    nc.tensor.matmul(pg1[:], lstmih_sb[:, 0:128], qs[:], start=True, stop=False)
                    nc.tensor.matmul(pg1[:], lstmhh_sb[:, 0:128], hl[:], start=False, stop=True)
                    pg2 = psA.tile([P, GPC], dt.float32, tag="psmall")
                    nc.tensor.matmul(pg2[:], lstmih_sb[:, 128:256], qs[:], start=True, stop=False)
                    nc.tensor.matmul(pg2[:], lstmhh_sb[:, 128:256], hl[:], start=False, stop=True)
                    s_i = cp.tile([DIM, GPC], dt.float32, tag="s_i")
                    nc.scalar.activation(s_i[:], pg1[0:64, :], AF.Sigmoid, bias=lb4[:, 0:1])
                    s_f = cp.tile([DIM, GPC], dt.float32, tag="s_f")
                    nc.scalar.activation(s_f[:], pg1[64:128, :], AF.Sigmoid, bias=lb4[:, 1:2])
                    tg = cp.tile([DIM, GPC], dt.float32, tag="tg")
                    nc.scalar.activation(tg[:], pg2[0:64, :], AF.Tanh, bias=lb4[:, 2:3])
                    so = cp.tile([DIM, GPC], dt.float32, tag="so")
                    nc.scalar.activation(so[:], pg2[64:128, :], AF.Sigmoid, bias=lb4[:, 3:4])
                    w1 = cp.tile([DIM, GPC], dt.float32, tag="w1")
                    nc.vector.tensor_mul(w1[:], s_f[:], cl[:])
                    w2 = cp.tile([DIM, GPC], dt.float32, tag="w2")
                    nc.vector.tensor_mul(w2[:], s_i[:], tg[:])
                    nc.vector.tensor_add(cl[:], w1[:], w2[:])
                    tcl = cp.tile([DIM, GPC], dt.float32, tag="tcl")
                    nc.scalar.activation(tcl[:], cl[:], AF.Tanh)
                    nc.vector.tensor_mul(hl[:], so[:], tcl[:])

                    # hl node-major (bf16)
                    pht = psA.tile([GPC, DIM], dt.float32, tag="psmall")
                    nc.tensor.transpose(pht[:], hl[:], ident[0:64, 0:64])
                    hlnm = cp.tile([GPC, DIM], dt.bfloat16, tag="hlnm")
                    nc.vector.tensor_copy(hlnm[:], pht[:])

                    # e per tile group
                    NG4 = (T + 3) // 4
                    for g in range(NG4):
                        tlo = 4 * g
                        thi = min(4 * g + 4, T)
                        nt = thi - tlo
                        ph = psA.tile([P, 4 * DIM], dt.float32, tag="phlb")
                        for k in range(nt):
                            t = tlo + k
                            nc.tensor.matmul(ph[:, 64 * k:64 * (k + 1)],
                                             onehotT_sb[:, 128 * t:128 * (t + 1)],
                                             hlnm[:], start=True, stop=True)
                        scr = cp.tile([P, 4 * DIM], dt.float32, tag="escr")
                        nc.vector.tensor_mul(
                            scr[:, :64 * nt].rearrange("p (t w) -> p t w", w=64),
                            out_nm[:].rearrange("p (t w) -> p t w", w=65)[:, tlo:thi, 0:64],
                            ph[:, :64 * nt].rearrange("p (t w) -> p t w", w=64))
                        nc.vector.tensor_reduce(
                            e_nm[:, tlo:thi],
                            scr[:, :64 * nt].rearrange("p (t w) -> p t w", w=64),
                            axis=AX.X, op=AL.add)

                    a_nm = cp.tile([P, T4], dt.float32, tag="a_nm")
                    nc.scalar.activation(a_nm[:], e_nm[:], AF.Exp)

                    # aout = a * out (bf16), col 64 = a
                    for t in range(T):
                        nc.vector.tensor_scalar_mul(
                            aout[:, 65 * t:65 * (t + 1)],
                            out_nm[:, 65 * t:65 * (t + 1)], a_nm[:, t:t + 1])

                    # r_vec + asum
                    prv = psB.tile([GPC, 65], dt.float32, tag="prv")
                    for t in range(T):
                        nc.tensor.matmul(prv[:], ohnm_sb[:, GPC * t:GPC * (t + 1)],
                                         aout[:, 65 * t:65 * (t + 1)],
                                         start=(t == 0), stop=(t == T - 1))
                    asum = cp.tile([GPC, 1], dt.float32, tag="asum")
                    nc.vector.tensor_scalar_add(asum[:], prv[:, 64:65], 1e-16)
                    rec = cp.tile([GPC, 1], dt.float32, tag="rec")
                    nc.vector.reciprocal(rec[:], asum[:])
                    rvn = cp.tile([GPC, DIM], dt.float32, tag="rvn")
                    nc.vector.tensor_scalar_mul(rvn[:], prv[:, 0:64], rec[:])

                    # q_star = [hl ; r_vec^T]
                    nc.vector.tensor_copy(qs[0:64, :], hl[:])
                    prt = psA.tile([DIM, GPC], dt.float32, tag="psmall")
                    nc.tensor.transpose(prt[:], rvn[:], ident[0:GPC, 0:GPC])
                    nc.vector.tensor_copy(qs[64:128, :], prt[:])

                pq = psA.tile([GPC, P], dt.float32, tag="psmall")
                nc.tensor.transpose(pq[:], qs[:], ident[:])
                qs_nm = cp.tile([GPC, P], dt.float32, tag="qs_nm")
                nc.vector.tensor_copy(qs_nm[:], pq[:])
                nc.sync.dma_start(out=qs_out_d[:], in_=qs_nm[:])

    nc.compile()
    return nc


# ---------------------------------------------------------------------------
# entry point

def kernel(**inputs):
    x = np.asarray(inputs["x"], f32)
    edge_index = np.asarray(inputs["edge_index"])
    batch = np.asarray(inputs["batch"])
    N = x.shape[0]

    meta = _preprocess(edge_index, batch, N)
    T, NPAD, BLK = meta['T'], meta['NPAD'], meta['BLK']
    canon = meta['canon']

    nc = _build(meta)

    # shared (replicated) inputs
    Wih = np.asarray(inputs['gru_w_ih'], f32)
    Whh = np.asarray(inputs['gru_w_hh'], f32)
    rz_lhsT = np.zeros((P, P), f32)
    rz_lhsT[0:64, 0:64] = Whh[0:64].T
    rz_lhsT[64:128, 0:64] = Wih[0:64].T
    rz_lhsT[0:64, 64:128] = Whh[64:128].T
    rz_lhsT[64:128, 64:128] = Wih[64:128].T
    n_lhsT = np.zeros((P, P), f32)
    n_lhsT[0:64, 0:64] = Whh[128:192].T      # hn out cols 0:64 from h rows
    n_lhsT[64:128, 64:128] = Wih[128:192].T  # in out cols 64:128 from m rows
    bih3 = np.asarray(inputs['gru_b_ih'], f32).reshape(3, DIM).T.copy()
    bhh3 = np.asarray(inputs['gru_b_hh'], f32).reshape(3, DIM).T.copy()
    lstmih = np.asarray(inputs['lstm_w_ih'], f32).T.copy()   # [128, 256]
    lstmhh = np.asarray(inputs['lstm_w_hh'], f32).T.copy()   # [64, 256]
    lbi4 = np.asarray(inputs['lstm_b_ih'], f32).reshape(4, DIM).T.copy()
    lbh4 = np.asarray(inputs['lstm_b_hh'], f32).reshape(4, DIM).T.copy()

    shared = {
        "lin0w": np.asarray(inputs['lin0_w'], f32).T.copy(),
        "lin0b": np.asarray(inputs['lin0_b'], f32).reshape(DIM, 1),
        "nn1w": np.asarray(inputs['nn1_w'], f32).reshape(DIM, 1),
        "nn1b": np.asarray(inputs['nn1_b'], f32).reshape(DIM, 1),
        "nn2wT": np.asarray(inputs['nn2_w'], f32).T.copy(),
        "nn2b": np.asarray(inputs['nn2_b'], f32).reshape(1, DIM * DIM),
        "convb": np.asarray(inputs['conv_b'], f32).reshape(DIM, 1),
        "rzlhsT": rz_lhsT, "nlhsT": n_lhsT, "bih3": bih3, "bhh3": bhh3,
        "lstmih": lstmih, "lstmhh": lstmhh, "lbi4": lbi4, "lbh4": lbh4,
    }

    batch64 = np.asarray(batch, np.int64)
    in_maps = []
    for c in range(C):
        m = canon[c] >= 0
        ids = canon[c][m]
        x_fm = np.zeros((F_IN, NPAD), f32)
        x_fm[:, m] = x[ids].T
        degs = meta['deg_canon'][c]
        inv_fm = np.zeros((1, NPAD), f32)
        inv_fm[0, m] = np.where(degs[m] > 0, 1.0 / np.maximum(degs[m], 1), 0.0)
        onehotT = np.zeros((GPC, NPAD), f32)
        onehotT[batch64[ids] - GPC * c, np.nonzero(m)[0]] = 1.0
        ohnm = np.zeros((T, 128, GPC), f32)
        ohnm.reshape(T * 128, GPC)[np.nonzero(m)[0], batch64[ids] - GPC * c] = 1.0
        ohnm = ohnm.transpose(1, 0, 2).reshape(128, T * GPC)
        im = dict(shared)
        im.update({
            "x_fm": x_fm, "inv_fm": inv_fm,
            "onehotT": onehotT.astype(bf16), "ohnm": ohnm.astype(bf16),
            "og16": _wrap_idx16(meta['og_idx'][c]),
            "sm16": _wrap_idx16(meta['slotmap'][c]),
        })
        in_maps.append(im)

    res = bass_utils.run_bass_kernel_spmd(nc, in_maps, core_ids=list(range(C)),
                                          trace=TRACE)
    if TRACE:
        print(f"HW exec time: {res.exec_time_ns} ns")

    q_star = np.concatenate([res.results[c]["qs_out"] for c in range(C)], axis=0)
    feat_map = np.zeros((N, DIM), f32)
    for c in range(C):
        m = canon[c] >= 0
        feat_map[canon[c][m]] = res.results[c]["fm_out"][m]
    return q_star, feat_map


# revision 14
# speedup vs baseline: 1.4148x; 1.4148x over previous
"""Trainium2 Bass kernel for nn_Encoder (MPNN: NNConv + GRU + Set2Set).

Self-contained: host-side integer preprocessing (sharding, canonical
orders, slot maps, exchange schedule) + an 8-core SPMD Bass/Tile kernel.

Key structure exploited: edge_attr is all-ones, so the edge-conditioned
weight matrix is identical for every edge and message passing collapses to
   agg = (segment_sum(out[src], dst) * inv_deg) @ W_e.
The segment_sum is computed with on-chip (GPSIMD) gathers over a
degree-bucketed canonical node order; the cross-core rows are exchanged
with one AllToAll per iteration.

Sharding: graphs (batch segments) are split 32-per-core; each core owns its
graphs' nodes. dim-64 parameters are replicated. Segment softmax (Set2Set)
is fully core-local via one-hot matmuls.
"""
import sys
import types
import numpy as np

# ---------------------------------------------------------------------------
# axon NTFF profile hook shim (lets trace=True work when caller requests it)
try:
    import antenv  # noqa: F401
    if 'antenv.axon_hooks' not in sys.modules:
        try:
            from trn_agent_boot.trn_boot import _ntff_profile_via_ctypes
            _hook = _ntff_profile_via_ctypes('/opt/axon/libaxon_pjrt.so')
        except Exception:
            _hook = None
        _m = types.ModuleType('antenv.axon_hooks')
        _m.get_axon_ntff_profile_hook = lambda: _hook
        _m.set_axon_ntff_profile_hook = lambda h: None
        sys.modules['antenv.axon_hooks'] = _m
except ImportError:
    pass

import ml_dtypes
import concourse.bass as bass
import concourse.mybir as mybir
from concourse import bacc
from concourse.bass import get_trn_type
from concourse.tile import TileContext
from concourse import bass_utils
from concourse.masks import make_identity

C = 8
B = 256
DIM = 64
F_IN = 32
GPC = B // C
P = 128
f32 = np.float32
bf16 = ml_dtypes.bfloat16


# ---------------------------------------------------------------------------
# host preprocessing (pure integer / layout work)

def _preprocess(edge_index, batch, n_nodes):
    src_g = np.asarray(edge_index[0], np.int64)
    dst_g = np.asarray(edge_index[1], np.int64)
    batch = np.asarray(batch, np.int64)
    N = n_nodes

    core_starts = np.searchsorted(batch, np.arange(0, B + 1, GPC))
    node_core = batch // GPC
    n_c = np.diff(core_starts)

    deg = np.zeros(N, np.int64)
    np.add.at(deg, dst_g, 1)

    T = int(np.ceil(n_c.max() / 128))
    NPAD = T * 128
    canon = -np.ones((C, NPAD), np.int64)
    pos_of = -np.ones(N, np.int64)
    for c in range(C):
        ids = np.arange(core_starts[c], core_starts[c + 1])
        ids = ids[np.argsort(deg[ids], kind="stable")]
        canon[c, NPAD - len(ids):] = ids
        pos_of[ids] = np.arange(NPAD - len(ids), NPAD)

    deg_canon = np.zeros((C, NPAD), np.int64)
    for c in range(C):
        m = canon[c] >= 0
        deg_canon[c, m] = deg[canon[c, m]]

    D_t = deg_canon.reshape(C, T, 128).max(axis=(0, 2))
    n_slots = int((128 * D_t).sum())
    slot_off = np.concatenate([[0], np.cumsum(128 * D_t)]).astype(np.int64)

    slot_src = -np.ones((C, n_slots), np.int64)
    for c in range(C):
        e_mask = node_core[dst_g] == c
        e_dst_col = pos_of[dst_g[e_mask]]
        e_src = src_g[e_mask]
        order = np.argsort(e_dst_col, kind="stable")
        e_dst_col = e_dst_col[order]
        e_src = e_src[order]
        jj = np.arange(len(e_dst_col)) - np.searchsorted(e_dst_col, e_dst_col)
        t = e_dst_col // 128
        p = e_dst_col % 128
        slot_src[c, slot_off[t] + jj * 128 + p] = e_src

    blocks = [[None] * C for _ in range(C)]
    for c in range(C):
        srcs = slot_src[c]
        srcs = srcs[srcs >= 0]
        for o in range(C):
            blocks[o][c] = np.unique(srcs[node_core[srcs] == o])
    BLK = int(max(len(blocks[o][c]) for o in range(C) for c in range(C)))
    BLK = ((BLK + 1 + 127) // 128) * 128

    # pads point at the zero row (row NPAD of the h mirror)
    og_idx = np.full((C, C * BLK), NPAD, np.int64)
    for o in range(C):
        for c in range(C):
            cols = pos_of[blocks[o][c]]
            og_idx[o, c * BLK:c * BLK + len(cols)] = cols

    ZERO_COL = BLK - 1   # last row of block 0 is always a zero row
    slotmap = np.full((C, n_slots), ZERO_COL, np.int64)
    for c in range(C):
        for o in range(C):
            blk = blocks[o][c]
            if len(blk) == 0:
                continue
            mask = slot_src[c] >= 0
            s = slot_src[c][mask]
            sel = node_core[s] == o
            ranks = np.searchsorted(blk, s[sel])
            slotmap[c, np.nonzero(mask)[0][sel]] = o * BLK + ranks

    # degree runs: list of (deg, t0, t1) covering tiles with equal D_t
    runs = []
    t0 = 0
    for t in range(1, T + 1):
        if t == T or D_t[t] != D_t[t0]:
            runs.append((int(D_t[t0]), t0, t))
            t0 = t

    return dict(
        T=T, NPAD=NPAD, BLK=BLK, n_slots=n_slots, D_t=D_t, slot_off=slot_off,
        canon=canon, pos_of=pos_of, deg_canon=deg_canon, og_idx=og_idx,
        slotmap=slotmap, ZERO_COL=ZERO_COL, runs=runs, core_starts=core_starts,
    )


def _wrap_idx16(idx, channels=128):
    """ap_gather index layout: element i -> [16*g + i%16, i//16], replicated
    across every 16-partition group."""
    n = len(idx)
    cols = (n + 15) // 16
    flat = np.zeros(cols * 16, np.int64)
    flat[:n] = idx
    base = flat.reshape(cols, 16).T.astype(np.int16)
    out = np.zeros((channels, cols), np.int16)
    for g in range(channels // 16):
        out[16 * g:16 * g + 16] = base
    return out


# ---------------------------------------------------------------------------
# kernel builder

DEBUG_TAPS = False
TRACE = False


def _build(meta):
    T, NPAD, BLK = meta['T'], meta['NPAD'], meta['BLK']
    runs, slot_off = meta['runs'], meta['slot_off']
    OGN = C * BLK
    RCVW = C * BLK + 4
    NCHUNK = (NPAD + 511) // 512
    dt = mybir.dt

    nc = bacc.Bacc(get_trn_type(), target_bir_lowering=False, debug=False,
                   num_devices=C)

    # ---- DRAM I/O ----
    di = lambda name, shape, d=dt.float32: nc.dram_tensor(name, shape, d, kind="ExternalInput")
    x_fm_d = di("x_fm", [F_IN, NPAD])
    inv_d = di("inv_fm", [1, NPAD])
    onehotT_d = di("onehotT", [GPC, NPAD], dt.bfloat16)
    ohnm_d = di("ohnm", [P, T * GPC], dt.bfloat16)
    lin0w_d = di("lin0w", [F_IN, DIM])
    lin0b_d = di("lin0b", [DIM, 1])
    nn1w_d = di("nn1w", [DIM, 1])
    nn1b_d = di("nn1b", [DIM, 1])
    nn2wT_d = di("nn2wT", [DIM, DIM * DIM])
    nn2b_d = di("nn2b", [1, DIM * DIM])
    convb_d = di("convb", [DIM, 1])
    rzlhsT_d = di("rzlhsT", [P, P])
    nlhsT_d = di("nlhsT", [P, P])
    bih3_d = di("bih3", [DIM, 3])
    bhh3_d = di("bhh3", [DIM, 3])
    lstmih_d = di("lstmih", [P, 4 * DIM])
    lstmhh_d = di("lstmhh", [DIM, 4 * DIM])
    lbi4_d = di("lbi4", [DIM, 4])
    lbh4_d = di("lbh4", [DIM, 4])
    og16_d = di("og16", [P, OGN // 16], dt.int16)
    sm16_d = di("sm16", [P, meta['n_slots'] // 16], dt.int16)

    qs_out_d = nc.dram_tensor("qs_out", [GPC, P], dt.float32, kind="ExternalOutput")
    fm_out_d = nc.dram_tensor("fm_out", [NPAD, DIM], dt.float32, kind="ExternalOutput")
    if DEBUG_TAPS:
        dbg_W_d = nc.dram_tensor("dbg_W", [DIM, DIM], dt.float32, kind="ExternalOutput")
        dbg_h0_d = nc.dram_tensor("dbg_h0", [DIM, NPAD], dt.float32, kind="ExternalOutput")
        dbg_S0_d = nc.dram_tensor("dbg_S0", [DIM, NPAD], dt.float32, kind="ExternalOutput")
        dbg_h1_d = nc.dram_tensor("dbg_h1", [DIM, NPAD], dt.float32, kind="ExternalOutput")


    rg = [list(range(C))]
    AF = mybir.ActivationFunctionType
    AX = mybir.AxisListType
    AL = mybir.AluOpType

    with TileContext(nc) as tc:
        with tc.tile_pool(name="persist", bufs=1) as pp, \
             tc.tile_pool(name="chunk", bufs=2) as cp, \
             tc.tile_pool(name="dram", bufs=1, space="DRAM") as dp:

            # ---------------- static loads ----------------
            def load(pool, d_ap, shape, dtype=dt.float32, tag=None):
                t = pool.tile(shape, dtype, tag=tag or d_ap.name)
                nc.sync.dma_start(out=t[:], in_=d_ap[:])
                return t

            # mh: [0:64] = h state (gather source), [64:128] = m scratch
            mh = pp.tile([P, NPAD], dt.float32, tag="mh")
            onehotT_sb = load(pp, onehotT_d, [GPC, NPAD], dt.bfloat16)
            ohnm_sb = load(pp, ohnm_d, [P, T * GPC], dt.bfloat16)
            lin0w_sb = load(pp, lin0w_d, [F_IN, DIM])
            lin0b_sb = load(pp, lin0b_d, [DIM, 1])
            nn1w_sb = load(pp, nn1w_d, [DIM, 1])
            nn1b_sb = load(pp, nn1b_d, [DIM, 1])
            rzlhsT_sb = load(pp, rzlhsT_d, [P, P])
            nlhsT_sb = load(pp, nlhsT_d, [P, P])
            bih3_sb = load(pp, bih3_d, [DIM, 3])
            bhh3_sb = load(pp, bhh3_d, [DIM, 3])
            lstmih_sb = load(pp, lstmih_d, [P, 4 * DIM])
            lstmhh_sb = load(pp, lstmhh_d, [DIM, 4 * DIM])
            lbi4_sb = load(pp, lbi4_d, [DIM, 4])
            lbh4_sb = load(pp, lbh4_d, [DIM, 4])
            og_sb = load(pp, og16_d, [P, OGN // 16], dt.int16)
            sm_sb = load(pp, sm16_d, [P, meta['n_slots'] // 16], dt.int16)
            zero_sb = pp.tile([P, DIM], dt.float32, tag="zero_sb")
            nc.vector.memset(zero_sb[:], 0.0)
            out_nm = pp.tile([P, T * 65], dt.float32, tag="out_nm")
            nc.vector.memset(
                out_nm[:].rearrange("p (t w) -> p t w", w=65)[:, :, 64:65], 1.0)
            h_dram = dp.tile([NPAD + P, DIM], dt.float32, tag="h_dram")
            nc.sync.dma_start(out=h_dram[NPAD:, :], in_=zero_sb[:])

            ident = pp.tile([P, P], dt.float32, tag="ident")
            make_identity(nc, ident[:])

            # convb at base partition 64 (conv relu writes mh[64:128])
            convb128 = pp.tile([P, 1], dt.float32, tag="convb128")
            nc.sync.dma_start(out=convb128[0:64, :], in_=convb_d[:])
            nc.sync.dma_start(out=convb128[64:128, :], in_=convb_d[:])

            # combined GRU biases (all base-0 [64, k])
            brz2 = pp.tile([DIM, 2], dt.float32, tag="brz2")
            nc.vector.tensor_add(brz2[:], bih3_sb[:, 0:2], bhh3_sb[:, 0:2])
            lb4 = pp.tile([DIM, 4], dt.float32, tag="lb4")
            nc.vector.tensor_add(lb4[:], lbi4_sb[:], lbh4_sb[:])

            W_sb = pp.tile([DIM, DIM], dt.float32, tag="W_sb")
            inv_bc = pp.tile([DIM, NPAD], dt.float32, tag="inv_bc")
            nc.sync.dma_start(out=inv_bc[0:1, :], in_=inv_d[:])
            nc.gpsimd.partition_broadcast(inv_bc[:], inv_bc[0:1, :], channels=DIM)

            # ---------------- init phase: W_e + lin0 ----------------
            with tc.tile_pool(name="init", bufs=1) as ip, \
                 tc.tile_pool(name="pinit", bufs=2, space="PSUM") as pip:
                x_sb = load(ip, x_fm_d, [F_IN, NPAD])
                nn2wT_sb = load(ip, nn2wT_d, [DIM, DIM * DIM])
                nn2b_sb = load(ip, nn2b_d, [1, DIM * DIM])
                hid = ip.tile([DIM, 1], dt.float32, tag="hid")
                nc.scalar.activation(hid[:], nn1w_sb[:], AF.Relu, bias=nn1b_sb[:, 0:1])
                wvec = ip.tile([1, DIM * DIM], dt.float32, tag="wvec")
                for k in range(8):
                    pw = pip.tile([1, 512], dt.float32, tag="pw")
                    nc.tensor.matmul(pw[:], hid[:],
                                     nn2wT_sb[:, 512 * k:512 * (k + 1)],
                                     start=True, stop=True)
                    nc.vector.tensor_add(wvec[:, 512 * k:512 * (k + 1)], pw[:],
                                         nn2b_sb[:, 512 * k:512 * (k + 1)])
                wbuf = dp.tile([DIM, DIM], dt.float32, tag="wbuf")
                nc.sync.dma_start(out=wbuf[:].rearrange("a b -> (a b)").unsqueeze(0), in_=wvec[:])
                nc.sync.dma_start(out=W_sb[:], in_=wbuf[:])
                if DEBUG_TAPS:
                    nc.sync.dma_start(out=dbg_W_d[:], in_=W_sb[:])

                for ck in range(NCHUNK):
                    sl = slice(512 * ck, min(512 * (ck + 1), NPAD))
                    pl = pip.tile([DIM, 512], dt.float32, tag="pw")
                    w = sl.stop - sl.start
                    nc.tensor.matmul(pl[:, :w], lin0w_sb[:], x_sb[:, sl],
                                     start=True, stop=True)
                    nc.scalar.activation(mh[0:64, sl], pl[:, :w], AF.Relu,
                                         bias=lin0b_sb[:, 0:1])

            if DEBUG_TAPS:
                nc.sync.dma_start(out=dbg_h0_d[:], in_=mh[0:64, :])
            # ---------------- 3 message-passing + GRU iterations ----------------
            SLC = meta['n_slots'] // 128
            OGC = OGN // 128
            D_t = meta['D_t']

            def mirror_h(psuml, last):
                # transpose h (feature-major) into out_nm tiles, then DMA the
                # node-major view to h_dram for the next staging gather
                for t in range(T):
                    pt = psuml.tile([P, DIM], dt.float32, tag="ptr")
                    nc.tensor.transpose(pt[:], mh[0:64, 128 * t:128 * (t + 1)],
                                        ident[0:64, 0:64])
                    nc.vector.tensor_copy(out_nm[:, 65 * t:65 * t + 64], pt[:])
                if not last:
                    nc.sync.dma_start(
                        out=h_dram[:NPAD, :].rearrange("(t p) w -> p t w", p=128),
                        in_=out_nm[:].rearrange("p (t w) -> p t w", w=65)[:, :, 0:64])

            with tc.tile_pool(name="work", bufs=1) as wp, \
                 tc.tile_pool(name="ploop", bufs=1, space="PSUM") as psp, \
                 tc.tile_pool(name="ptrp", bufs=2, space="PSUM") as ptp:
                mirror_h(ptp, False)
                for it in range(3):
                    staged = wp.tile([P, OGC * DIM], dt.float32, tag="staged")
                    nc.gpsimd.dma_gather(
                        out_ap=staged[:].rearrange("p (c e) -> p c e", e=DIM),
                        in_ap=h_dram[:],
                        idxs_ap=og_sb[:],
                        num_idxs=OGN, num_idxs_reg=OGN,
                        elem_size=DIM, single_packet=False,
                    )
                    a2a_in = dp.tile([C, BLK, DIM], dt.float32, tag="a2a_in")
                    nc.sync.dma_start(
                        out=a2a_in[:].rearrange("c b w -> (c b) w").rearrange(
                            "(g p) w -> p g w", p=128),
                        in_=staged[:].rearrange("p (c e) -> p c e", e=DIM))
                    a2a_out = dp.tile([C, BLK, DIM], dt.float32, tag=f"a2a_out{it}")
                    nc.gpsimd.collective_compute(
                        "AllToAll", AL.bypass, replica_groups=rg,
                        ins=[a2a_in[:]], outs=[a2a_out[:]],
                    )
                    M = wp.tile([P, SLC * DIM], dt.float32, tag="Mslots")
                    nc.gpsimd.dma_gather(
                        out_ap=M[:].rearrange("p (c e) -> p c e", e=DIM),
                        in_ap=a2a_out[:].rearrange("c b w -> (c b) w"),
                        idxs_ap=sm_sb[:],
                        num_idxs=meta['n_slots'], num_idxs_reg=meta['n_slots'],
                        elem_size=DIM, single_packet=False,
                    )
                    # per-degree-run reduce (j-major slots) -> S_nm then transpose
                    S_nm = wp.tile([P, T * DIM], dt.float32, tag="S_nm")
                    for (dg, t0, t1) in runs:
                        osl = slice(DIM * t0, DIM * t1)
                        if dg == 0:
                            nc.vector.memset(S_nm[:, osl], 0.0)
                            continue
                        cb0 = int(slot_off[t0]) // 128
                        nt = t1 - t0
                        mv = M[:, cb0 * DIM:(cb0 + nt * dg) * DIM].rearrange(
                            "p (t j w) -> p t w j", j=dg, w=DIM)
                        if dg == 1:
                            nc.vector.tensor_copy(
                                S_nm[:, osl],
                                M[:, cb0 * DIM:(cb0 + nt) * DIM])
                        else:
                            nc.vector.tensor_reduce(
                                S_nm[:, osl].rearrange("p (t w) -> p t w", w=DIM),
                                mv, axis=AX.X, op=AL.add)
                    S = wp.tile([DIM, NPAD], dt.float32, tag="S")
                    for t in range(T):
                        ps_ = ptp.tile([DIM, P], dt.float32, tag="pst")
                        nc.tensor.transpose(ps_[:], S_nm[:, DIM * t:DIM * (t + 1)],
                                            ident[:])
                        nc.vector.tensor_copy(S[:, 128 * t:128 * (t + 1)], ps_[:])

                    if DEBUG_TAPS and it == 0:
                        nc.sync.dma_start(out=dbg_S0_d[:], in_=S[:])
                    # conv + GRU per 512-chunk
                    for ck in range(NCHUNK):
                        sl = slice(512 * ck, min(512 * (ck + 1), NPAD))
                        w = sl.stop - sl.start
                        pc = psp.tile([DIM, 512], dt.float32, tag="pconv")
                        nc.tensor.matmul(pc[:, :w], W_sb[:], S[:, sl],
                                         start=True, stop=True)
                        pc2 = psp.tile([DIM, 512], dt.float32, tag="pconv2")
                        nc.vector.tensor_mul(pc2[:, :w], pc[:, :w], inv_bc[:, sl])
                        nc.scalar.activation(mh[64:128, sl], pc2[:, :w], AF.Relu,
                                             bias=convb128[64:128, :])
                        prz = psp.tile([P, 512], dt.float32, tag="prz")
                        nc.tensor.matmul(prz[:, :w], rzlhsT_sb[:], mh[:, sl],
                                         start=True, stop=True)
                        pn = psp.tile([P, 512], dt.float32, tag="pn")
                        nc.tensor.matmul(pn[:, :w], nlhsT_sb[:], mh[:, sl],
                                         start=True, stop=True)
                        r_t = cp.tile([DIM, 512], dt.float32, tag="r_t")
                        nc.scalar.activation(r_t[:, :w], prz[0:64, :w], AF.Sigmoid,
                                             bias=brz2[:, 0:1])
                        z_t = cp.tile([DIM, 512], dt.float32, tag="z_t")
                        nc.scalar.activation(z_t[:, :w], prz[64:128, :w], AF.Sigmoid,
                                             bias=brz2[:, 1:2])
                        hn_t = cp.tile([DIM, 512], dt.float32, tag="hn_t")
                        nc.scalar.activation(hn_t[:, :w], pn[0:64, :w], AF.Identity,
                                             bias=bhh3_sb[:, 2:3])
                        in_t = cp.tile([DIM, 512], dt.float32, tag="in_t")
                        nc.scalar.activation(in_t[:, :w], pn[64:128, :w], AF.Identity,
                                             bias=bih3_sb[:, 2:3])
                        rn = cp.tile([DIM, 512], dt.float32, tag="rn")
                        nc.vector.tensor_mul(rn[:, :w], r_t[:, :w], hn_t[:, :w])
                        nc.vector.tensor_add(rn[:, :w], rn[:, :w], in_t[:, :w])
                        ng = cp.tile([DIM, 512], dt.float32, tag="ng")
                        nc.scalar.activation(ng[:, :w], rn[:, :w], AF.Tanh)
                        dd = cp.tile([DIM, 512], dt.float32, tag="dd")
                        nc.vector.tensor_sub(dd[:, :w], mh[0:64, sl], ng[:, :w])
                        nc.vector.tensor_mul(dd[:, :w], z_t[:, :w], dd[:, :w])
                        nc.vector.tensor_add(mh[0:64, sl], ng[:, :w], dd[:, :w])
                    if DEBUG_TAPS and it == 0:
                        nc.sync.dma_start(out=dbg_h1_d[:], in_=mh[0:64, :])
                    mirror_h(ptp, it == 2)

            # ---------------- Set2Set ----------------
            with tc.tile_pool(name="s2s", bufs=1) as sp, \
                 tc.tile_pool(name="ps2sA", bufs=2, space="PSUM") as psA, \
                 tc.tile_pool(name="ps2sB", bufs=1, space="PSUM") as psB:

                nc.sync.dma_start(
                    out=fm_out_d[:].rearrange("(t p) w -> p t w", p=128),
                    in_=out_nm[:].rearrange("p (t w) -> p t w", w=65)[:, :, 0:64])

                hl = sp.tile([DIM, GPC], dt.float32, tag="hl")
                cl = sp.tile([DIM, GPC], dt.float32, tag="cl")
                qs = sp.tile([P, GPC], dt.float32, tag="qs")
                nc.vector.memset(hl[:], 0.0)
                nc.vector.memset(cl[:], 0.0)
                nc.vector.memset(qs[:], 0.0)
                T4 = ((T + 3) // 4) * 4
                e_nm = sp.tile([P, T4], dt.float32, tag="e_nm")
                nc.vector.memset(e_nm[:], 0.0)
                aout = sp.tile([P, T * 65], dt.bfloat16, tag="aout")

                for step in range(3):
                    # LSTM cell
                    pg1 = psA.tile([P, GPC], dt.float32, tag="psmall")
                    nc.tensor.matmul(pg1[:], lstmih_sb[:, 0:128], qs[:], start=True, stop=False)
                    nc.tensor.matmul(pg1[:], lstmhh_sb[:, 0:128], hl[:], start=False, stop=True)
                    pg2 = psA.tile([P, GPC], dt.float32, tag="psmall")
                    nc.tensor.matmul(pg2[:], lstmih_sb[:, 128:256], qs[:], start=True, stop=False)
                    nc.tensor.matmul(pg2[:], lstmhh_sb[:, 128:256], hl[:], start=False, stop=True)
                    s_i = cp.tile([DIM, GPC], dt.float32, tag="s_i")
                    nc.scalar.activation(s_i[:], pg1[0:64, :], AF.Sigmoid, bias=lb4[:, 0:1])
                    s_f = cp.tile([DIM, GPC], dt.float32, tag="s_f")
                    nc.scalar.activation(s_f[:], pg1[64:128, :], AF.Sigmoid, bias=lb4[:, 1:2])
                    tg = cp.tile([DIM, GPC], dt.float32, tag="tg")
                    nc.scalar.activation(tg[:], pg2[0:64, :], AF.Tanh, bias=lb4[:, 2:3])
                    so = cp.tile([DIM, GPC], dt.float32, tag="so")
                    nc.scalar.activation(so[:], pg2[64:128, :], AF.Sigmoid, bias=lb4[:, 3:4])
                    w1 = cp.tile([DIM, GPC], dt.float32, tag="w1")
                    nc.vector.tensor_mul(w1[:], s_f[:], cl[:])
                    w2 = cp.tile([DIM, GPC], dt.float32, tag="w2")
                    nc.vector.tensor_mul(w2[:], s_i[:], tg[:])
                    nc.vector.tensor_add(cl[:], w1[:], w2[:])
                    tcl = cp.tile([DIM, GPC], dt.float32, tag="tcl")
                    nc.scalar.activation(tcl[:], cl[:], AF.Tanh)
                    nc.vector.tensor_mul(hl[:], so[:], tcl[:])

                    # hl node-major (bf16)
                    pht = psA.tile([GPC, DIM], dt.float32, tag="psmall")
                    nc.tensor.transpose(pht[:], hl[:], ident[0:64, 0:64])
                    hlnm = cp.tile([GPC, DIM], dt.bfloat16, tag="hlnm")
                    nc.vector.tensor_copy(hlnm[:], pht[:])

                    # e per tile group
                    NG4 = (T + 3) // 4
                    for g in range(NG4):
                        tlo = 4 * g
                        thi = min(4 * g + 4, T)
                        nt = thi - tlo
                        ph = psA.tile([P, 4 * DIM], dt.float32, tag="phlb")
                        for k in range(nt):
                            t = tlo + k
                            nc.tensor.matmul(ph[:, 64 * k:64 * (k + 1)],
                                             onehotT_sb[:, 128 * t:128 * (t + 1)],
                                             hlnm[:], start=True, stop=True)
                        scr = cp.tile([P, 4 * DIM], dt.float32, tag="escr")
                        nc.vector.tensor_mul(
                            scr[:, :64 * nt].rearrange("p (t w) -> p t w", w=64),
                            out_nm[:].rearrange("p (t w) -> p t w", w=65)[:, tlo:thi, 0:64],
                            ph[:, :64 * nt].rearrange("p (t w) -> p t w", w=64))
                        nc.vector.tensor_reduce(
                            e_nm[:, tlo:thi],
                            scr[:, :64 * nt].rearrange("p (t w) -> p t w", w=64),
                            axis=AX.X, op=AL.add)

                    a_nm = cp.tile([P, T4], dt.float32, tag="a_nm")
                    nc.scalar.activation(a_nm[:], e_nm[:], AF.Exp)

                    # aout = a * out (bf16), col 64 = a
                    for t in range(T):
                        nc.vector.tensor_scalar_mul(
                            aout[:, 65 * t:65 * (t + 1)],
                            out_nm[:, 65 * t:65 * (t + 1)], a_nm[:, t:t + 1])

                    # r_vec + asum
                    prv = psB.tile([GPC, 65], dt.float32, tag="prv")
                    for t in range(T):
                        nc.tensor.matmul(prv[:], ohnm_sb[:, GPC * t:GPC * (t + 1)],
                                         aout[:, 65 * t:65 * (t + 1)],
                                         start=(t == 0), stop=(t == T - 1))
                    asum = cp.tile([GPC, 1], dt.float32, tag="asum")
                    nc.vector.tensor_scalar_add(asum[:], prv[:, 64:65], 1e-16)
                    rec = cp.tile([GPC, 1], dt.float32, tag="rec")
                    nc.vector.reciprocal(rec[:], asum[:])
                    rvn = cp.tile([GPC, DIM], dt.float32, tag="rvn")
                    nc.vector.tensor_scalar_mul(rvn[:], prv[:, 0:64], rec[:])

                    # q_star = [hl ; r_vec^T]
                    nc.vector.tensor_copy(qs[0:64, :], hl[:])
                    prt = psA.tile([DIM, GPC], dt.float32, tag="psmall")
                    nc.tensor.transpose(prt[:], rvn[:], ident[0:GPC, 0:GPC])
                    nc.vector.tensor_copy(qs[64:128, :], prt[:])

                pq = psA.tile([GPC, P], dt.float32, tag="psmall")
                nc.tensor.transpose(pq[:], qs[:], ident[:])
                qs_nm = cp.tile([GPC, P], dt.float32, tag="qs_nm")
                nc.vector.tensor_copy(qs_nm[:], pq[:])
                nc.sync.dma_start(out=qs_out_d[:], in_=qs_nm[:])

    nc.compile()
    return nc


# ---------------------------------------------------------------------------
# entry point

def kernel(**inputs):
    x = np.asarray(inputs["x"], f32)
    edge_index = np.asarray(inputs["edge_index"])
    batch = np.asarray(inputs["batch"])
    N = x.shape[0]

    meta = _preprocess(edge_index, batch, N)
    T, NPAD, BLK = meta['T'], meta['NPAD'], meta['BLK']
    canon = meta['canon']

    nc = _build(meta)

    # shared (replicated) inputs
    Wih = np.asarray(inputs['gru_w_ih'], f32)
    Whh = np.asarray(inputs['gru_w_hh'], f32)
    rz_lhsT = np.zeros((P, P), f32)
    rz_lhsT[0:64, 0:64] = Whh[0:64].T
    rz_lhsT[64:128, 0:64] = Wih[0:64].T
    rz_lhsT[0:64, 64:128] = Whh[64:128].T
    rz_lhsT[64:128, 64:128] = Wih[64:128].T
    n_lhsT = np.zeros((P, P), f32)
    n_lhsT[0:64, 0:64] = Whh[128:192].T      # hn out cols 0:64 from h rows
    n_lhsT[64:128, 64:128] = Wih[128:192].T  # in out cols 64:128 from m rows
    bih3 = np.asarray(inputs['gru_b_ih'], f32).reshape(3, DIM).T.copy()
    bhh3 = np.asarray(inputs['gru_b_hh'], f32).reshape(3, DIM).T.copy()
    lstmih = np.asarray(inputs['lstm_w_ih'], f32).T.copy()   # [128, 256]
    lstmhh = np.asarray(inputs['lstm_w_hh'], f32).T.copy()   # [64, 256]
    lbi4 = np.asarray(inputs['lstm_b_ih'], f32).reshape(4, DIM).T.copy()
    lbh4 = np.asarray(inputs['lstm_b_hh'], f32).reshape(4, DIM).T.copy()

    shared = {
        "lin0w": np.asarray(inputs['lin0_w'], f32).T.copy(),
        "lin0b": np.asarray(inputs['lin0_b'], f32).reshape(DIM, 1),
        "nn1w": np.asarray(inputs['nn1_w'], f32).reshape(DIM, 1),
        "nn1b": np.asarray(inputs['nn1_b'], f32).reshape(DIM, 1),
        "nn2wT": np.asarray(inputs['nn2_w'], f32).T.copy(),
        "nn2b": np.asarray(inputs['nn2_b'], f32).reshape(1, DIM * DIM),
        "convb": np.asarray(inputs['conv_b'], f32).reshape(DIM, 1),
        "rzlhsT": rz_lhsT, "nlhsT": n_lhsT, "bih3": bih3, "bhh3": bhh3,
        "lstmih": lstmih, "lstmhh": lstmhh, "lbi4": lbi4, "lbh4": lbh4,
    }

    batch64 = np.asarray(batch, np.int64)
    in_maps = []
    for c in range(C):
        m = canon[c] >= 0
        ids = canon[c][m]
        x_fm = np.zeros((F_IN, NPAD), f32)
        x_fm[:, m] = x[ids].T
        degs = meta['deg_canon'][c]
        inv_fm = np.zeros((1, NPAD), f32)
        inv_fm[0, m] = np.where(degs[m] > 0, 1.0 / np.maximum(degs[m], 1), 0.0)
        onehotT = np.zeros((GPC, NPAD), f32)
        onehotT[batch64[ids] - GPC * c, np.nonzero(m)[0]] = 1.0
        ohnm = np.zeros((T, 128, GPC), f32)
        ohnm.reshape(T * 128, GPC)[np.nonzero(m)[0], batch64[ids] - GPC * c] = 1.0
        ohnm = ohnm.transpose(1, 0, 2).reshape(128, T * GPC)
        im = dict(shared)
        im.update({
            "x_fm": x_fm, "inv_fm": inv_fm,
            "onehotT": onehotT.astype(bf16), "ohnm": ohnm.astype(bf16),
            "og16": _wrap_idx16(meta['og_idx'][c]),
            "sm16": _wrap_idx16(meta['slotmap'][c]),
        })
        in_maps.append(im)

    res = bass_utils.run_bass_kernel_spmd(nc, in_maps, core_ids=list(range(C)),
                                          trace=TRACE)
    if TRACE:
        print(f"HW exec time: {res.exec_time_ns} ns")

    q_star = np.concatenate([res.results[c]["qs_out"] for c in range(C)], axis=0)
    feat_map = np.zeros((N, DIM), f32)
    for c in range(C):
        m = canon[c] >= 0
        feat_map[canon[c][m]] = res.results[c]["fm_out"][m]
    return q_star, feat_map
